# revision 1
# baseline (speedup 1.0000x reference)
"""Self-contained Trainium2 Bass kernel for nn_BipartiteGNN (collapsed linear
form, local_scatter architecture).

The network is fully linear, so the [1,1] output collapses to degree-chain
vectors d = A^T 1, p = A^T d', z = A^T p' per side, contracted with X.
The device computes d/p/z and the contractions Y = [mask,d,p,z] @ [X|1] on 8
NeuronCores (out-side nodes sharded 12500/core); the host runs only index
preprocessing (edge->slot assignment) and the tiny 64-dim weight recursion on
the per-core [4,65] outputs.

Per chain pass, per core: the gathered-side vector u (bf16, AllGathered
between passes) is broadcast-DMAed into all partitions as local_scatter
*data*; host-static int16 idx tensors route each edge's value into fixed
5-slot runs per (node, u-chunk); strided DVE reduces and partition combines
produce per-node totals. Contested data columns are served from a hot column
region filled by a small on-device ap_gather from the same data tile.
"""

import numpy as np

CH = 12544
CLS = 32
WIN = 392
S = 5
COLS = WIN * S            # 1960
CW = CH // 2              # 6272 chunk width
NCALL = 8
NCHUNK = 16
HOT = 512
DW = CW + HOT             # 6528 data width
NREAL = 12500
NCORE = 8
NNODE = 100000


def node_uid(node, side):
    """Global node id (out of 100000) -> flat u position."""
    core = node // NREAL
    l = node - core * NREAL
    return core * 2 * CH + side * CH + (l % CLS) * WIN + l // CLS


def slot_uid(core, l, side):
    return core * 2 * CH + side * CH + (l % CLS) * WIN + l // CLS


def assign_direction(dcore, dl, gu, gside):
    """Assign edges. dcore/dl: out-side owner core + local slot (0..CH).
    gu: flat uid of the gather-side value. Returns per-edge placement plus
    hot lists and overflow mask."""
    n = len(dl)
    cls = dl // WIN
    win = dl % WIN

    gowner = gu // (2 * CH)
    g_in_owner = gu - gowner * 2 * CH - gside * CH
    assert (g_in_owner >= 0).all() and (g_in_owner < CH).all()
    sc = gowner * 2 + (g_in_owner >= CW)
    datacol = g_in_owner % CW
    call = sc % NCALL
    h = sc // NCALL

    # rank within (dcore, dl, sc) group
    key1 = ((dcore * np.int64(CH) + dl) * NCHUNK + sc)
    order = np.argsort(key1, kind="stable")
    k_s = key1[order]
    first = np.ones(n, bool)
    first[1:] = k_s[1:] != k_s[:-1]
    gstart = np.zeros(n, np.int64)
    gstart[first] = np.arange(n)[first]
    gstart = np.maximum.accumulate(gstart)
    rank = np.arange(n) - gstart
    r = np.empty(n, np.int64)
    r[order] = rank

    over = r >= 2 * S
    q = (r % 2).astype(np.int64)
    slot = r // 2
    cur_q = np.where(over, 0, q)
    cur_slot = np.where(over, 0, slot)
    cur_p = cls + 32 * cur_q + 64 * h
    cur_col = win * S + cur_slot
    hot_edges = np.zeros(n, bool)
    act = ~over

    # occupancy of (key1, q, slot)
    occ_set = set(((key1[act] * 2 + cur_q[act]) * S + cur_slot[act]).tolist())

    idxs = np.arange(n)
    for rnd in range(5):
        k2 = ((dcore * NCALL + call) * 128 + cur_p) * np.int64(CW) + datacol
        dummy = ~act | hot_edges
        k2[dummy] = -1 - idxs[dummy]
        orderc = np.argsort(k2, kind="stable")
        k2s = k2[orderc]
        firstc = np.ones(n, bool)
        firstc[1:] = k2s[1:] != k2s[:-1]
        is_extra = np.zeros(n, bool)
        is_extra[orderc] = ~firstc
        is_extra &= act & ~hot_edges
        n_extra = int(is_extra.sum())
        if n_extra == 0:
            break
        if rnd == 4:
            hot_edges |= is_extra
            break
        ex_idx = idxs[is_extra]
        for e in ex_idx:
            q2 = 1 - cur_q[e]
            placed = False
            base = key1[e] * 2
            for s2 in range(S):
                k = (base + q2) * S + s2
                if k not in occ_set:
                    occ_set.discard((base + cur_q[e]) * S + cur_slot[e])
                    occ_set.add(k)
                    cur_q[e] = q2
                    cur_slot[e] = s2
                    cur_p[e] = cls[e] + 32 * q2 + 64 * h[e]
                    cur_col[e] = win[e] * S + s2
                    placed = True
                    break
            if not placed:
                hot_edges[e] = True

    # hot list construction: entries keyed by (dcore, call, datacol, dup-rank)
    hotlist = [[[] for _ in range(NCALL)] for _ in range(NCORE)]
    hotpos_of_edge = np.full(n, -1, np.int64)
    he = idxs[hot_edges & act]
    if len(he):
        kk = (((dcore[he] * NCALL + call[he]) * np.int64(CW) + datacol[he]) * 128
              + cur_p[he])
        oo = np.argsort(kk, kind="stable")
        kks = kk[oo]
        ff = np.ones(len(he), bool)
        ff[1:] = kks[1:] != kks[:-1]
        gs = np.zeros(len(he), np.int64)
        gs[ff] = np.arange(len(he))[ff]
        gs = np.maximum.accumulate(gs)
        prank = np.arange(len(he)) - gs
        prank_e = np.empty(len(he), np.int64)
        prank_e[oo] = prank
        # one hot entry per (dcore, call, pair=datacol//2, prank); the
        # on-device ap_gather is d=2 over bf16, so each entry delivers the
        # whole pair and both parities share it.
        pair = datacol[he] // 2
        hk = (((dcore[he] * NCALL + call[he]) * np.int64(CW // 2) + pair) * 16
              + prank_e)
        uniq, inv = np.unique(hk, return_inverse=True)
        u_pair = (uniq // 16) % (CW // 2)
        u_call = (uniq // 16 // (CW // 2)) % NCALL
        u_core = uniq // 16 // (CW // 2) // NCALL
        pos = np.zeros(len(uniq), np.int64)
        for c in range(NCORE):
            for j in range(NCALL):
                m = (u_core == c) & (u_call == j)
                cnt = int(m.sum())
                if cnt > HOT // 2:
                    raise RuntimeError(f"hot overflow core {c} call {j}: {cnt}")
                pos[m] = np.arange(cnt)
                hotlist[c][j] = u_pair[m].tolist()
        hotpos_of_edge[he] = pos[inv]

    final_datacol = np.where(
        hot_edges, CW + 2 * hotpos_of_edge + datacol % 2, datacol)
    return dict(n=n, call=call, p=cur_p, col=cur_col, datacol=final_datacol,
                over=over, hotlist=hotlist)


def build_arrays(dcore, a):
    """idx/vm arrays from an assignment (excluding overflow edges)."""
    idx_arr = np.full((NCORE, NCALL, 128, DW), -1, np.int16)
    vm_arr = np.zeros((NCORE, NCALL, 128, COLS), np.float32)
    m = ~a["over"]
    c0 = dcore[m]; c1 = a["call"][m]; c2 = a["p"][m]
    c3 = a["datacol"][m]; c4 = a["col"][m]
    k_dc = ((c0 * NCALL + c1) * 128 + c2) * np.int64(DW) + c3
    if len(np.unique(k_dc)) != len(k_dc):
        raise RuntimeError("datacol conflicts remain")
    k_tc = ((c0 * NCALL + c1) * 128 + c2) * np.int64(COLS) + c4
    if len(np.unique(k_tc)) != len(k_tc):
        raise RuntimeError("target col conflicts remain")
    idx_arr[c0, c1, c2, c3] = c4.astype(np.int16)
    vm_arr[c0, c1, c2, c4] = 1.0
    return idx_arr, vm_arr


def wrap_hot(hotlist):
    """hotlist[core][call] = list of pair indices -> ap_gather idx
    [NCORE, NCALL, 128, HOT//2//16] int16 (same list shared by all 8 Q7
    cores; d=2 gather delivers value pairs into the hot region)."""
    out = np.zeros((NCORE, NCALL, 128, HOT // 2 // 16), np.int16)
    for c in range(NCORE):
        for j in range(NCALL):
            hl = hotlist[c][j]
            for i, dc in enumerate(hl):
                part = i % 16
                colp = i // 16
                for k in range(8):
                    out[c, j, 16 * k + part, colp] = dc
    return out


def build_side(row_d_nodes, gu, gside, extra_d=None, extra_gu=None):
    """Build one direction. row_d_nodes: out-side global node ids.
    gu: gather-side flat uids. extra_*: spare-read duplicate edges
    (dslot given as (core, local)). Returns arrays + overflow list."""
    dcore = row_d_nodes // NREAL
    dl_ = row_d_nodes - dcore * NREAL
    dl = (dl_ % CLS) * WIN + dl_ // CLS
    if extra_d is not None and len(extra_d[0]):
        dcore = np.concatenate([dcore, extra_d[0]])
        dl = np.concatenate([dl, extra_d[1]])
        gu = np.concatenate([gu, extra_gu])
    a = assign_direction(dcore, dl, gu, gside)
    idx_arr, vm_arr = build_arrays(dcore, a)
    hot = wrap_hot(a["hotlist"])
    om = a["over"]
    return idx_arr, vm_arr, hot, (dcore[om], dl[om], gu[om])


def slot_flat_arr(l):
    return (l % CLS) * WIN + l // CLS


def full_prepare(E_A, E_B, verbose=False):
    """Iterated build with spares. Returns per-direction arrays + spare maps.
    Direction A: out-side s (side 0), gathers t (side 1). B: out t, gathers s."""
    def init(E, gside):
        d = E[0]
        dcore = d // NREAL
        dl = slot_flat_arr(d - dcore * NREAL)
        gu = node_uid(E[1], gside)
        return dcore, dl, gu
    dAc, dAl, guA = init(E_A, 1)
    dBc, dBl, guB = init(E_B, 0)
    spare_next = [[NREAL] * NCORE for _ in range(2)]
    spares = [[], []]  # per side: (core, spare_slot, parent_slot)
    for it in range(8):
        aA = assign_direction(dAc, dAl, guA, 1)
        aB = assign_direction(dBc, dBl, guB, 0)
        nA, nB = int(aA["over"].sum()), int(aB["over"].sum())
        if verbose:
            print(f"prep iter {it}: overflow A={nA} B={nB}", flush=True)
        if nA == 0 and nB == 0:
            idxA, vmA = build_arrays(dAc, aA)
            idxB, vmB = build_arrays(dBc, aB)
            hotA = wrap_hot(aA["hotlist"])
            hotB = wrap_hot(aB["hotlist"])
            hcA = [max(len(aA["hotlist"][c][j]) for c in range(NCORE))
                   for j in range(NCALL)]
            hcB = [max(len(aB["hotlist"][c][j]) for c in range(NCORE))
                   for j in range(NCALL)]
            return dict(idxA=idxA, vmA=vmA, hotA=hotA,
                        idxB=idxB, vmB=vmB, hotB=hotB, spares=spares,
                        hcA=hcA, hcB=hcB,
                        dA=(dAc, dAl, guA), dB=(dBc, dBl, guB))
        new_dupA = ([], [], [])
        new_dupB = ([], [], [])
        for side, a, dc_arr, dl_arr, gu_arr in ((0, aA, dAc, dAl, guA),
                                                (1, aB, dBc, dBl, guB)):
            ov = np.nonzero(a["over"])[0]
            for e in ov:
                core = int(dc_arr[e]); psl = int(dl_arr[e])
                lsp = spare_next[side][core]
                spare_next[side][core] += 1
                if lsp >= CH:
                    raise RuntimeError("out of spare slots")
                sp_slot = (lsp % CLS) * WIN + lsp // CLS
                spares[side].append((core, sp_slot, psl))
                dl_arr[e] = sp_slot
                sp_uid = core * 2 * CH + side * CH + sp_slot
                parent_uid = core * 2 * CH + side * CH + psl
                if side == 0:
                    mask = guB == parent_uid
                    new_dupB[0].append(dBc[mask]); new_dupB[1].append(dBl[mask])
                    new_dupB[2].append(np.full(int(mask.sum()), sp_uid))
                else:
                    mask = guA == parent_uid
                    new_dupA[0].append(dAc[mask]); new_dupA[1].append(dAl[mask])
                    new_dupA[2].append(np.full(int(mask.sum()), sp_uid))
        if new_dupA[0]:
            dAc = np.concatenate([dAc] + new_dupA[0])
            dAl = np.concatenate([dAl] + new_dupA[1])
            guA = np.concatenate([guA] + new_dupA[2])
        if new_dupB[0]:
            dBc = np.concatenate([dBc] + new_dupB[0])
            dBl = np.concatenate([dBl] + new_dupB[1])
            guB = np.concatenate([guB] + new_dupB[2])
    raise RuntimeError("spare iteration did not converge")


def build_inputs(P, x_s, x_t):
    """Per-core in_maps for the device kernel."""
    import ml_dtypes
    BF = ml_dtypes.bfloat16
    s_arr = np.arange(CH)
    cls = s_arr // WIN
    win = s_arr % WIN
    l_of_slot = win * CLS + cls
    valid = l_of_slot < NREAL

    def pack_x(x, side):
        out = np.zeros((NCORE, CH, 65), np.float32)
        out[:, :, 64] = 1.0
        for c in range(NCORE):
            out[c, valid, :64] = x[c * NREAL + l_of_slot[valid]]
        for (c, sp_slot, parent_slot) in P["spares"][side]:
            pw, pc = parent_slot % WIN, parent_slot // WIN
            pl = pw * CLS + pc
            out[c, sp_slot, :64] = x[c * NREAL + pl]
        return out.reshape(NCORE, 128, NCOL98, 65)

    NCOL98_ = CH // 128
    global NCOL98
    NCOL98 = NCOL98_
    xs_in = pack_x(np.asarray(x_s, np.float32), 0)
    xt_in = pack_x(np.asarray(x_t, np.float32), 1)
    mx = np.where(valid, 1.0, 0.0).astype(np.float32).reshape(128, NCOL98)

    in_maps = []
    for c in range(NCORE):
        in_maps.append({
            "idxA": P["idxA"][c], "idxB": P["idxB"][c],
            "hotA": P["hotA"][c], "hotB": P["hotB"][c],
            "vmA": P["vmA"][c].astype(BF), "vmB": P["vmB"][c].astype(BF),
            "xs": xs_in[c], "xt": xt_in[c],
            "mxs": mx, "mxt": mx,
        })
    return in_maps



from contextlib import ExitStack
import concourse.bass as bass
import concourse.tile as tile
from concourse import bacc, mybir
from concourse.bass_utils import run_bass_kernel_spmd

F32 = mybir.dt.float32
I16 = mybir.dt.int16
BF16 = mybir.dt.bfloat16

NCOL98 = CH // 128  # 98  (kept: used by kernel + input packing)


def build_kernel(hcA=None, hcB=None):
    # hcA/hcB: per-call hot-entry counts (max over cores), from prep; None
    # falls back to the full HOT//2 capacity.
    if hcA is None:
        hcA = [HOT // 2] * NCALL
    if hcB is None:
        hcB = [HOT // 2] * NCALL
    n16A = [min(16, (c + 15) // 16) for c in hcA]
    n16B = [min(16, (c + 15) // 16) for c in hcB]
    nc = bacc.Bacc("TRN2", target_bir_lowering=False, debug=False, num_devices=8)

    def din(name, shape, dt):
        return nc.dram_tensor(name, shape, dt, kind="ExternalInput")

    ins = {
        "idxA": din("idxA", [NCALL, 128, DW], I16),
        "idxB": din("idxB", [NCALL, 128, DW], I16),
        "hotA": din("hotA", [NCALL, 128, HOT // 2 // 16], I16),
        "hotB": din("hotB", [NCALL, 128, HOT // 2 // 16], I16),
        "vmA": din("vmA", [NCALL, 128, COLS], BF16),
        "vmB": din("vmB", [NCALL, 128, COLS], BF16),
        "xs": din("xs", [128, NCOL98, 65], F32),
        "xt": din("xt", [128, NCOL98, 65], F32),
        "mxs": din("mxs", [128, NCOL98], F32),
        "mxt": din("mxt", [128, NCOL98], F32),
    }
    res_s = nc.dram_tensor("res_s", [4, 65], F32, kind="ExternalOutput")
    res_t = nc.dram_tensor("res_t", [4, 65], F32, kind="ExternalOutput")

    dp_loc = nc.dram_tensor("dp_loc", [2 * CH], BF16)
    pp_loc = nc.dram_tensor("pp_loc", [2 * CH], BF16)
    d_full = nc.dram_tensor("d_full", [16 * CH], BF16, addr_space="Shared")
    p_full = nc.dram_tensor("p_full", [16 * CH], BF16, addr_space="Shared")
    dloc_f = nc.dram_tensor("dloc_f", [2 * CH], F32)
    ploc_f = nc.dram_tensor("ploc_f", [2 * CH], F32)
    zloc_f = nc.dram_tensor("zloc_f", [2 * CH], F32)

    with tile.TileContext(nc) as tc, ExitStack() as ctx:
        dpool = ctx.enter_context(tc.tile_pool(name="dp", bufs=3))
        ospool = ctx.enter_context(tc.tile_pool(name="osp", bufs=3))
        accp = ctx.enter_context(tc.tile_pool(name="accs", bufs=1))
        psum = ctx.enter_context(tc.tile_pool(name="ps", bufs=1, space="PSUM"))
        cpool = ctx.enter_context(tc.tile_pool(name="cp", bufs=1))

        def combine(acc, tag):
            # DVE ops need operands on the same partition range; realign the
            # upper partition blocks with SBUF->SBUF DMAs before adding.
            t64 = ospool.tile([64, WIN], F32, tag="t64", name=f"t64{tag}")
            nc.sync.dma_start(t64[:], acc[64:128, :])
            nc.vector.tensor_tensor(acc[0:64, :], acc[0:64, :], t64[:],
                                    mybir.AluOpType.add)
            t32 = ospool.tile([32, WIN], F32, tag="t32", name=f"t32{tag}")
            nc.sync.dma_start(t32[:], acc[32:64, :])
            nc.vector.tensor_tensor(acc[0:32, :], acc[0:32, :], t32[:],
                                    mybir.AluOpType.add)

        def publish(acc, bf_dram, bf_off, f_dram, f_off, tag):
            if bf_dram is not None:
                bft = ospool.tile([32, WIN], BF16, tag="pub", name=f"pub{tag}")
                nc.vector.tensor_copy(bft[:], acc[0:32, :])
                nc.sync.dma_start(
                    bass.AP(bf_dram, bf_off, [[WIN, 32], [1, WIN]]), bft[:])
            nc.sync.dma_start(
                bass.AP(f_dram, f_off, [[WIN, 32], [1, WIN]]), acc[0:32, :])

        def phase_d():
            for side, vm_in, f_off in ((0, ins["vmA"], 0), (1, ins["vmB"], CH)):
                acc = accp.tile([128, WIN], F32, tag=f"accd{side}",
                                name=f"accd{side}")
                for j in range(NCALL):
                    vmt = dpool.tile([128, COLS], BF16, tag="vm", name="vmt")
                    nc.sync.dma_start(vmt[:], vm_in[j, :, :])
                    red = ospool.tile([128, WIN], F32, tag="red", name="redd")
                    nc.vector.tensor_reduce(
                        red[:],
                        bass.AP(vmt.tensor, 0, [[COLS, 128], [S, WIN], [1, S]]),
                        mybir.AxisListType.X, mybir.AluOpType.add)
                    if j == 0:
                        nc.vector.tensor_copy(acc[:], red[:])
                    else:
                        nc.vector.tensor_tensor(acc[:], acc[:], red[:],
                                                mybir.AluOpType.add)
                combine(acc, f"d{side}")
                publish(acc, dp_loc, f_off, dloc_f, f_off, f"d{side}")

        def pass_table(idx_in, hot_in, n16, gside, u_full_dram,
                       bf_dram, bf_off, f_dram, f_off, tag):
            acc = accp.tile([128, WIN], F32, tag=f"acc{tag}", name=f"acc{tag}")
            for j in range(NCALL):
                nhot = 16 * n16[j]          # hot columns used = 2*nhot
                wj = CW + 2 * nhot          # scatter sweep width this call
                data = dpool.tile([128, DW], BF16, tag="data", name="datat")
                for half, sc in ((0, j), (1, j + 8)):
                    owner, hh = sc // 2, sc % 2
                    off = owner * 2 * CH + gside * CH + hh * CW
                    nc.sync.dma_start(
                        data[64 * half:64 * (half + 1), 0:CW],
                        bass.AP(u_full_dram, off, [[0, 64], [1, CW]]))
                if nhot:
                    hott = dpool.tile([128, HOT // 2 // 16], I16, tag="hot",
                                      name="hott")
                    nc.sync.dma_start(hott[:, 0:n16[j]],
                                      hot_in[j, :, 0:n16[j]])
                    nc.gpsimd.ap_gather(data[:, CW:wj], data[:, 0:CW],
                                        hott[:, 0:n16[j]],
                                        channels=128, num_elems=CW // 2, d=2,
                                        num_idxs=nhot)
                idxt = dpool.tile([128, DW], I16, tag="idx", name="idxt")
                nc.sync.dma_start(idxt[:, 0:wj], idx_in[j, :, 0:wj])
                os = ospool.tile([128, COLS], BF16, tag="os", name="ost")
                nc.gpsimd.local_scatter(os[:], data[:, 0:wj], idxt[:, 0:wj],
                                        channels=128,
                                        num_elems=COLS, num_idxs=wj)
                red = ospool.tile([128, WIN], F32, tag="red", name="redt")
                nc.vector.tensor_reduce(
                    red[:],
                    bass.AP(os.tensor, 0, [[COLS, 128], [S, WIN], [1, S]]),
                    mybir.AxisListType.X, mybir.AluOpType.add)
                if j == 0:
                    nc.vector.tensor_copy(acc[:], red[:])
                else:
                    nc.vector.tensor_tensor(acc[:], acc[:], red[:],
                                            mybir.AluOpType.add)
            combine(acc, tag)
            publish(acc, bf_dram, bf_off, f_dram, f_off, tag)

        def allgather(loc, full):
            nc.gpsimd.collective_compute(
                "AllGather", mybir.AluOpType.bypass,
                replica_groups=[list(range(8))],
                ins=[bass.AP(loc, 0, [[1, 1], [1, 2 * CH]]).opt()],
                outs=[bass.AP(full, 0, [[1, 1], [1, 16 * CH]]).opt()])

        phase_d()
        allgather(dp_loc, d_full)
        # p_s: out-side s (dir A), gathers t-side d values
        pass_table(ins["idxA"], ins["hotA"], n16A, 1, d_full, pp_loc, 0, ploc_f, 0, "ps")
        pass_table(ins["idxB"], ins["hotB"], n16B, 0, d_full, pp_loc, CH, ploc_f, CH, "pt")
        allgather(pp_loc, p_full)
        pass_table(ins["idxA"], ins["hotA"], n16A, 1, p_full, None, 0, zloc_f, 0, "zs")
        pass_table(ins["idxB"], ins["hotB"], n16B, 0, p_full, None, 0, zloc_f, CH, "zt")

        # contraction per side (side s issued right after z_s so its DVE/PE
        # work overlaps z_t's scatters)
        for side, xin, mxin, off, rout in ((0, "xs", "mxs", 0, res_s),
                                           (1, "xt", "mxt", CH, res_t)):
            u4 = cpool.tile([128, NCOL98, 4], F32, tag="u4", name=f"u4{side}")
            nc.sync.dma_start(
                bass.AP(u4.tensor, 0, [[NCOL98 * 4, 128], [4, NCOL98], [1, 1]]),
                ins[mxin].ap())
            for i, dram in enumerate((dloc_f, ploc_f, zloc_f)):
                nc.sync.dma_start(
                    bass.AP(u4.tensor, i + 1,
                            [[NCOL98 * 4, 128], [4, NCOL98], [1, 1]]),
                    bass.AP(dram, off, [[NCOL98, 128], [1, NCOL98]]))
            xr = cpool.tile([128, NCOL98, 65], F32, tag="xr", name=f"xr{side}")
            nc.sync.dma_start(xr[:], ins[xin].ap())
            ps = psum.tile([4, 65], F32, tag="psc", name=f"psc{side}")
            for m in range(NCOL98):
                nc.tensor.matmul(ps[:], u4[:, m, :], xr[:, m, :],
                                 start=(m == 0), stop=(m == NCOL98 - 1))
            outt = ospool.tile([4, 65], F32, tag="outt", name=f"outt{side}")
            nc.vector.tensor_copy(outt[:], ps[:])
            nc.sync.dma_start(rout.ap(), outt[:])

    nc.compile()
    return nc


def final_recursion(Ys, Yt, Ss, St, inputs):
    """Ys/Yt: [4, 64] weighted sums (rows: 1, d, p, z). Ss/St: [4] sums."""
    f64 = np.float64
    Wl_s2t = np.asarray(inputs["Wl_s2t"], f64); Wr_s2t = np.asarray(inputs["Wr_s2t"], f64)
    b_s2t = np.asarray(inputs["b_s2t"], f64)
    Wl_t2s = np.asarray(inputs["Wl_t2s"], f64); Wr_t2s = np.asarray(inputs["Wr_t2s"], f64)
    b_t2s = np.asarray(inputs["b_t2s"], f64)
    W_lin = np.asarray(inputs["W_lin"], f64); b_lin = np.asarray(inputs["b_lin"], f64)
    Ys = Ys.astype(f64); Yt = Yt.astype(f64)
    Ss = Ss.astype(f64); St = St.astype(f64)
    L = 3

    def term(side, u_id, r, layer):
        if layer == 0:
            Y = Ys if side == "s" else Yt
            return Y[u_id] @ r
        if side == "s":
            Wl, Wr, b, Sv, other = Wl_t2s[layer-1], Wr_t2s[layer-1], b_t2s[layer-1], Ss, "t"
        else:
            Wl, Wr, b, Sv, other = Wl_s2t[layer-1], Wr_s2t[layer-1], b_s2t[layer-1], St, "s"
        return (term(other, u_id + 1, Wl @ r, layer - 1)
                + Sv[u_id] * (b @ r)
                + term(side, u_id, Wr @ r, layer - 1))

    r0 = W_lin[:, 0]
    tot = term("s", 0, r0, L) + term("t", 0, r0, L) + b_lin[0]
    return np.array([[tot]], dtype=np.float32)


def simulate_expected(P, inputs):
    """Numerically mirror the device pipeline (bf16 tables) in numpy."""
    import ml_dtypes
    BF = ml_dtypes.bfloat16
    NU = NCORE * 2 * CH

    def totals_from_edges(dtup, u_full_bf):
        dcore, dslot, gu = dtup
        t = np.zeros((NCORE, CH), np.float32)
        np.add.at(t, (dcore, dslot), u_full_bf[gu].astype(np.float32))
        return t

    def counts(dtup):
        dcore, dslot, gu = dtup
        t = np.zeros((NCORE, CH), np.float32)
        np.add.at(t, (dcore, dslot), 1.0)
        return t

    def publish(t_s, t_t):
        full = np.zeros(NU, BF)
        for c in range(NCORE):
            full[c*2*CH:c*2*CH+CH] = t_s[c].astype(BF)
            full[c*2*CH+CH:(c+1)*2*CH] = t_t[c].astype(BF)
        return full

    d_s = counts(P["dA"]); d_t = counts(P["dB"])
    d_full = publish(d_s, d_t)
    p_s = totals_from_edges(P["dA"], d_full)
    p_t = totals_from_edges(P["dB"], d_full)
    p_full = publish(p_s, p_t)
    z_s = totals_from_edges(P["dA"], p_full)
    z_t = totals_from_edges(P["dB"], p_full)

    in_maps = build_inputs(P, inputs["x_s"], inputs["x_t"])
    Ys = np.zeros((4, 65)); Yt = np.zeros((4, 65))
    for c in range(NCORE):
        for (Y, xkey, mkey, dd, pp, zz) in ((Ys, "xs", "mxs", d_s, p_s, z_s),
                                            (Yt, "xt", "mxt", d_t, p_t, z_t)):
            x65 = np.asarray(in_maps[c][xkey], np.float32).reshape(CH, 65)
            mx = np.asarray(in_maps[c][mkey], np.float32).reshape(CH)
            u4 = np.stack([mx, dd[c], pp[c], zz[c]])
            Y += u4 @ x65
    return final_recursion(Ys[:, :64], Yt[:, :64], Ys[:, 64], Yt[:, 64], inputs)


_NC_CACHE = {}


def hot_counts(P):
    """Per-call hot-entry counts (max over cores), recorded at prep time."""
    return P["hcA"], P["hcB"]


def kernel(**inputs):
    E_A = np.asarray(inputs["edges_s2t"], np.int64)
    E_B = np.asarray(inputs["edges_t2s"], np.int64)
    P = full_prepare(E_A, E_B)
    in_maps = build_inputs(P, inputs["x_s"], inputs["x_t"])
    hcA, hcB = hot_counts(P)
    key = (tuple(hcA), tuple(hcB))
    if key not in _NC_CACHE:
        _NC_CACHE[key] = build_kernel(hcA, hcB)
    nc = _NC_CACHE[key]
    res = run_bass_kernel_spmd(nc, in_maps, core_ids=list(range(8)), trace=False)
    Ys = sum(r["res_s"] for r in res.results).astype(np.float64)
    Yt = sum(r["res_t"] for r in res.results).astype(np.float64)
    return final_recursion(Ys[:, :64], Yt[:, :64], Ys[:, 64], Yt[:, 64], inputs)

